# revision 33
# baseline (speedup 1.0000x reference)
"""Causal single-head attention on 8 trn2 NeuronCores.

B=4, S=2048, D_MODEL=1024, D_HEAD=64, fp32 in/out.

Sharding: 2 cores per batch. Core half h=0 owns query tiles {0..3,12..15}
(rows 0:512, 1536:2048), h=1 owns {4..11} (rows 512:1536); both own 68
causal 128x128 blocks. The host feeds each core its batch's embeddings
already TRANSPOSED to E^T [dm, s] in bf16 with columns permuted so own
query rows come first - no on-device transposes/casts of E at all.

Per-core pipeline (identical SPMD program):
  Projections from E^T with packed weights: own chunks use [Wq/8|Wk]
  (M=128, full PE array) plus V; other chunks use [Wk|Wv]. Outputs land
  in a stacked QKT sbuf [128, S] (rows 0:64 Q^T, 64:128 K^T). V tiles are
  PE-transposed into Vp [128k, 16, 65] with a ones column (denominator).
  Attention over local key tiles kt:
    kt 0..3  : scores vs both slots (N=1024 via 2 matmuls into one 2-bank
               PSUM), one exp, tri-mask multiply on slot0 cols
    kt 4..7  : slot1 only (N=512), tri mask
    kt 8..11 : both slots; slot0 multiplied by per-core 0/1 gate vector
    kt 12..15: slot1 only; per-core 0/-30000 exp bias kills it on h=1
  PV accumulates out^T [65, 512] per slot in PSUM (col 64 = sum exp);
  host does the final divide + transpose + scatter.
"""

import sys

if "/opt/trn_rl_repo" not in sys.path:
    sys.path.insert(0, "/opt/trn_rl_repo")

import numpy as np

B, S, D, H = 4, 2048, 1024, 64
P = 128
KO = D // P          # 8 dmodel chunks
NT = S // P          # 16 seq tiles
NEG = -30000.0


def _halves():
    return [[(0, 512), (1536, 2048)], [(512, 1536)]]


def _build_program(zb):
    import concourse.bacc as bacc
    import concourse.mybir as mybir
    import concourse.tile as tile

    f32 = mybir.dt.float32
    bf16 = mybir.dt.bfloat16
    AF = mybir.ActivationFunctionType
    ALU = mybir.AluOpType

    nc = bacc.Bacc()
    # et layout [chunk, partition, KO*512]: 8 KB contiguous per partition
    # per chunk -> big DMA descriptors (1 KB descriptors run ~21 GB/s/queue)
    et = nc.declare_dram_parameter("et", [4, P, KO * 512], bf16, isOutput=False)
    # weights + ET chunk 0 fused into one DMA: per partition
    # cols 0:1536 = [Wv|Wk|Wq/8] x 8 ko (192 each), cols 1536:5632 = chunk0
    wc0 = nc.declare_dram_parameter("wc0", [P, 1536 + 4096], bf16, isOutput=False)
    # cols: bq/8 | bk | g8 | g12n | bv (bv only rows 0:64 meaningful)
    bias4 = nc.declare_dram_parameter("bias4", [P, 5], f32, isOutput=False)
    # cols 0:2048 = tri masks (4 x 512), cols 2048:2112 = identity (rows 0:64)
    mi = nc.declare_dram_parameter("mi", [P, 4 * 512 + H], bf16, isOutput=False)
    out = nc.declare_dram_parameter("out", [H + 1, 1024], f32, isOutput=True)

    from contextlib import ExitStack

    with tile.TileContext(nc) as tc, ExitStack() as ctx:
        cpool = ctx.enter_context(tc.tile_pool(name="const", bufs=1))
        vtp = ctx.enter_context(tc.tile_pool(name="vt", bufs=2))
        ptp = ctx.enter_context(tc.tile_pool(name="pt", bufs=8))
        psb = ctx.enter_context(tc.tile_pool(name="psb", bufs=2, space="PSUM"))

        # --- input DMAs. Each dma_start spreads across all 16 queues at
        # ~300 GB/s aggregate but pays ~0.5us fixed cost, and queues come
        # up staggered over the first ~7us - so: few dma_starts, the
        # startup-critical ones (weights, chunk0 sub-chunks) first.
        wc_sb = cpool.tile([P, 1536 + 4096], bf16, tag="wc0")
        # two halves: subtile deps let the first projection matmuls start
        # once the first half (weights + chunk0 ko 0..4) has landed
        nc.sync.dma_start(wc_sb[:, 0:4096], wc0[:, 0:4096])
        nc.sync.dma_start(wc_sb[:, 4096:5632], wc0[:, 4096:5632])
        # [partition, chunk, ko, 512]; chunk 0 lives in wc_sb instead
        ET = cpool.tile([P, 4, KO, 512], bf16, tag="ET")
        nc.sync.dma_start(ET[:, 1, :, :], et[1, :, :])

        def w_ap(ko, a, b):      # weight cols a:b of ko-th 192-block
            return wc_sb[:, ko * 192 + a:ko * 192 + b]

        def et_ap(cc, ko):       # ET chunk cc, ko-th 512-col block
            if cc == 0:
                return wc_sb[:, 1536 + ko * 512:1536 + (ko + 1) * 512]
            return ET[:, cc, ko, :]
        bias_sb = cpool.tile([P, 5], f32, tag="bias4")
        nc.sync.dma_start(bias_sb[:], bias4[:])
        mi_sb = cpool.tile([P, 4 * 512 + H], bf16, tag="mi")
        nc.sync.dma_start(mi_sb[:], mi[:])
        nc.sync.dma_start(ET[:, 2, :, :], et[2, :, :])
        nc.sync.dma_start(ET[:, 3, :, :], et[3, :, :])
        bq_sb = bias_sb[:, 0:1]
        bk_sb = bias_sb[:, 1:2]
        g8_sb = bias_sb[:, 2:3]
        g12_sb = bias_sb[:, 3:4]
        bv_sb = bias_sb[:H, 4:5]
        id_sb = mi_sb[:H, 4 * 512:4 * 512 + H]

        def mask_ap(j):
            return mi_sb[:, j * 512:(j + 1) * 512]

        # Q^T and K^T both live on partitions 64:128 (matmul requires lhsT
        # and rhs to share a base partition; the packed [Wv|Wk] projection
        # puts K^T on PSUM rows 64:128 and DVE copies cannot shift rows).
        QT = cpool.tile([P, 1024], bf16, tag="QT")
        KT = cpool.tile([P, S], bf16, tag="KT")
        Vp = cpool.tile([P, NT, H + 1], bf16, tag="Vp")
        nc.vector.memset(Vp[:, :, H:H + 1], 1.0)
        o_sb = cpool.tile([P, 1024], f32, tag="osb")

        def vtranspose(vt, cc):
            for t in range(4):
                kt = cc * 4 + t
                pvt = psb.tile([P, H], bf16, tag="pj", name=f"pvt_{kt}")
                nc.tensor.transpose(
                    pvt[:], vt[:, t * P:(t + 1) * P], id_sb[:]
                )
                nc.vector.tensor_copy(Vp[:, kt, :H], pvt[:])

        vts = [None] * 4

        def pcopy(dst, src_ap, bias, on_act):
            # PSUM->SBUF projection copy; with zero biases the early copies
            # run on the ACT engine, which is otherwise idle until the
            # first exp (~15us later)
            if zb and on_act:
                nc.scalar.activation(dst, src_ap, AF.Copy)
            else:
                nc.vector.tensor_scalar_add(dst, src_ap, bias)

        def vk_chunk(cc):
            # one pass of the ET chunk computes V^T (rows 0:64) + K^T (64:128)
            ps = psb.tile([P, 512], f32, tag="pj", name=f"vk_ps_{cc}")
            for ko in range(KO):
                nc.tensor.matmul(
                    ps[:], w_ap(ko, 0, 128), et_ap(cc, ko),
                    start=(ko == 0), stop=(ko == KO - 1),
                )
            on_act = cc < 2
            pcopy(
                KT[H:P, cc * 512:(cc + 1) * 512], ps[H:P, :], bk_sb[H:P],
                on_act,
            )
            vt = vtp.tile([H, 512], bf16, tag="vt", name=f"vt_{cc}")
            pcopy(vt[:], ps[:H, :], bv_sb[:], on_act)
            vts[cc] = vt

        def q_chunk(cc):
            # M=64 matmul targeting PSUM rows 64:128 so Q^T lands at base 64
            ps = psb.tile([P, 512], f32, tag="pj", name=f"q_ps_{cc}")
            for ko in range(KO):
                nc.tensor.matmul(
                    ps[H:P, :], w_ap(ko, 128, 192), et_ap(cc, ko),
                    start=(ko == 0), stop=(ko == KO - 1),
                )
            pcopy(
                QT[H:P, cc * 512:(cc + 1) * 512], ps[H:P, :], bq_sb[H:P],
                True,
            )

        # --- attention ---
        outT0 = psb.tile([P, 512], f32, tag="os0", bufs=1)
        outT1 = psb.tile([P, 512], f32, tag="os1", bufs=1)
        pts = [None] * NT

        # pvs[kt] = list of (outT, col0, rhs_ap) PV pieces for that key tile
        pvs = [None] * NT

        def sc(kt):
            # score regions trimmed to the causal need:
            #  kt 0..3  : cols [kt*128 : 1024] (slot0 tri tail + slot1 full)
            #  kt 4..7  : slot1 tri tail, cols [(kt-4)*128 : 512] of slot1
            #  kt 8..11 : both slots full; slot0 multiplied by 0/1 gate
            #  kt 12..15: slot1 full, exp-bias gated
            ps = psb.tile(
                [P, 1024], f32, tag="sc", name=f"sc_{kt}", bufs=2
            )
            kblk = KT[H:P, kt * P:(kt + 1) * P]
            pt = ptp.tile([P, 1024], bf16, tag="pt", name=f"pt_{kt}")
            if kt < 4 or (8 <= kt < 12):
                c0 = kt * P if kt < 4 else 0
                nc.tensor.matmul(
                    ps[:, c0:512], kblk, QT[H:P, c0:512],
                    start=True, stop=True, skip_group_check=True,
                )
                nc.tensor.matmul(
                    ps[:, 512:1024], kblk, QT[H:P, 512:1024],
                    start=True, stop=True, skip_group_check=True,
                )
                nc.scalar.activation(pt[:, c0:1024], ps[:, c0:1024], AF.Exp)
                if kt < 4:
                    if c0 < 512:
                        nc.vector.tensor_tensor(
                            pt[:, c0:512], pt[:, c0:512],
                            mi_sb[:, kt * 512 + c0:(kt + 1) * 512], ALU.mult
                        )
                else:
                    nc.vector.tensor_scalar_mul(
                        pt[:, 0:512], pt[:, 0:512], g8_sb[:]
                    )
                pvs[kt] = [
                    (outT0, c0, pt[:, c0:512]),
                    (outT1, 0, pt[:, 512:1024]),
                ]
            else:
                c0 = (kt - 4) * P if kt < 12 else 0
                n = 512 - c0
                nc.tensor.matmul(
                    ps[:, 0:n], kblk, QT[H:P, 512 + c0:1024],
                    start=True, stop=True, skip_group_check=True,
                )
                if kt >= 12:
                    nc.scalar.activation(
                        pt[:, 0:n], ps[:, 0:n], AF.Exp, bias=g12_sb[:]
                    )
                else:
                    nc.scalar.activation(pt[:, 0:n], ps[:, 0:n], AF.Exp)
                    nc.vector.tensor_tensor(
                        pt[:, 0:n], pt[:, 0:n],
                        mi_sb[:, (kt - 4) * 512 + c0:(kt - 3) * 512], ALU.mult
                    )
                pvs[kt] = [(outT1, c0, pt[:, 0:n])]

        def pv(kt):
            for outT, c0, rhs in pvs[kt]:
                nc.tensor.matmul(
                    outT[:H + 1, c0:512], Vp[:, kt, :], rhs,
                    start=(kt == 0),
                    stop=(kt == 11 if outT is outT0 else kt == 15),
                    skip_group_check=True,
                )

        # --- emission order = per-engine FIFO order; hand-pipelined so PE
        # never waits on ACT/DVE and ACT starts exping early ---
        # HAM warmup: the PE sequencer comes alive ~4us before the first
        # input DMA lands; dependency-free matmuls on an uninitialized
        # scratch tile open the clock gate (1.2 -> 2.4 GHz) in that window
        # so the projections run warm. Results go to a dead psum tile.
        wtile = cpool.tile([P, P], bf16, tag="warm")
        nc.vector.memset(wtile[:], 0.0)
        for i in range(30):
            wps = psb.tile([P, H], f32, tag="pj", name=f"warm_{i}")
            nc.tensor.matmul(
                wps[:], wtile[:], wtile[:, 0:H],
                start=True, stop=True, skip_group_check=True,
            )

        vk_chunk(0)
        q_chunk(0)
        q_chunk(1)
        sc(0)
        sc(1)
        sc(2)
        sc(3)
        vk_chunk(1)
        sc(4)
        sc(5)
        vtranspose(vts[0], 0)
        sc(6)
        sc(7)
        vtranspose(vts[1], 1)
        pv(0)
        pv(1)
        pv(2)
        pv(3)
        vk_chunk(2)
        sc(8)
        pv(4)
        sc(9)
        pv(5)
        vk_chunk(3)
        vtranspose(vts[2], 2)
        sc(10)
        pv(6)
        sc(11)
        pv(7)
        sc(12)
        pv(8)
        sc(13)
        pv(9)
        vtranspose(vts[3], 3)
        sc(14)
        pv(10)
        sc(15)
        pv(11)
        nc.vector.tensor_copy(o_sb[:H + 1, 0:512], outT0[:H + 1, :])
        nc.sync.dma_start(out[:, 0:512], o_sb[:H + 1, 0:512])
        pv(12)
        pv(13)
        pv(14)
        pv(15)
        nc.vector.tensor_copy(o_sb[:H + 1, 512:1024], outT1[:H + 1, :])
        nc.sync.dma_start(out[:, 512:1024], o_sb[:H + 1, 512:1024])

    nc.finalize()
    return nc


_CACHED = None


def _get_program(zb):
    global _CACHED
    if _CACHED is None or _CACHED[0] != zb:
        _CACHED = (zb, _build_program(zb))
    return _CACHED[1]


def _host_inputs(embeddings, Wq, bq, Wk, bk, Wv, bv):
    import ml_dtypes

    bf16 = ml_dtypes.bfloat16
    halves = _halves()
    # multiplicative tri masks, [k, j, c] layout: 1 where c >= k + j*128
    masks = np.zeros((P, 4, 512), np.float32)
    for j in range(4):
        for k in range(P):
            masks[k, j, k + j * P:] = 1.0
    ident = np.zeros((P, H), np.float32)
    ident[:H] = np.eye(H, dtype=np.float32)
    mi = np.ascontiguousarray(
        np.concatenate([masks.reshape(P, 4 * 512), ident], axis=1)
    ).astype(bf16)

    def wlay(w):
        return np.asarray(w, np.float32).reshape(KO, P, H).transpose(1, 0, 2)

    wq8l = wlay(Wq) / 8.0
    wkl = wlay(Wk)
    wvl = wlay(Wv)
    wts = np.concatenate([wvl, wkl, wq8l], axis=2).reshape(P, 1536)
    bqf = np.asarray(bq, np.float32) / 8.0
    bkf = np.asarray(bk, np.float32)
    bvf = np.asarray(bv, np.float32)
    z64 = np.zeros(H, np.float32)
    bq8P = np.concatenate([z64, bqf])
    bkP = np.concatenate([z64, bkf])
    bvP = np.concatenate([bvf, z64])

    in_maps = []
    perms = []
    for c in range(8):
        b, h = c // 2, c % 2
        own = halves[h]
        other = halves[1 - h]
        rows = np.concatenate(
            [np.arange(a, z) for a, z in own] + [np.arange(a, z) for a, z in other]
        )
        perms.append(rows)
        ep = embeddings[b][rows]                      # [S, D] f32, permuted
        etl = np.ascontiguousarray(
            ep.T.reshape(KO, P, 4, 512).transpose(2, 1, 0, 3)
        ).astype(bf16).reshape(4, P, KO * 512)        # [cc, p, ko*512]
        g8v = np.full(P, 1.0 if h == 1 else 0.0, np.float32)
        g12v = np.full(P, NEG if h == 1 else 0.0, np.float32)
        bias4 = np.ascontiguousarray(
            np.stack([bq8P, bkP, g8v, g12v, bvP], axis=1)
        )
        wc0l = np.ascontiguousarray(
            np.concatenate([wts, etl[0]], axis=1)
        ).astype(bf16)
        in_maps.append({
            "et": etl, "wc0": wc0l, "bias4": bias4, "mi": mi,
        })
    return in_maps, perms


def _run(embeddings, Wq, bq, Wk, bk, Wv, bv, trace=False):
    from concourse.bass_utils import run_bass_kernel_spmd

    zb = (
        not np.any(np.asarray(bq)) and not np.any(np.asarray(bk))
        and not np.any(np.asarray(bv))
    )
    nc = _get_program(zb)
    in_maps, perms = _host_inputs(embeddings, Wq, bq, Wk, bk, Wv, bv)
    res = run_bass_kernel_spmd(
        nc, in_maps, core_ids=list(range(8)), trace=trace,
        trace_cores=list(range(8)) if trace else None,
    )
    full = np.empty((B, S, H), np.float32)
    for c in range(8):
        b = c // 2
        o = res.results[c]["out"]                     # [65, 1024] f32
        full[b, perms[c][:1024]] = (o[:H] / o[H:H + 1]).T
    return full, res


def kernel(embeddings, Wq, bq, Wk, bk, Wv, bv):
    full, _ = _run(
        np.asarray(embeddings, np.float32), Wq, bq, Wk, bk, Wv, bv, trace=False
    )
    return full


# revision 55
# speedup vs baseline: 1.0143x; 1.0143x over previous
"""Causal single-head attention on 8 trn2 NeuronCores.

B=4, S=2048, D_MODEL=1024, D_HEAD=64, fp32 in/out.

Sharding: 2 cores per batch. Core half h=0 owns query tiles {0..3,12..15}
(rows 0:512, 1536:2048), h=1 owns {4..11} (rows 512:1536); both own 68
causal 128x128 blocks. The host feeds each core its batch's embeddings
already TRANSPOSED to E^T [dm, s] in bf16 with columns permuted so own
query rows come first - no on-device transposes/casts of E at all.

Per-core pipeline (identical SPMD program, all matmuls bf16):
  Warmup matmuls on a scratch tile open the PE HAM clock gate
  (1.2->2.4 GHz) while the first input DMA is in flight. Inputs move in
  few large dma_starts (each internally spans all 16 DMA queues).
  Projections per 512-col chunk of E^T: one [Wv|Wk]-packed pass (V^T on
  PSUM rows 0:64, K^T on rows 64:128) plus, for the core's own 2 chunks,
  a Wq/8 pass targeting PSUM rows 64:128. Q^T and K^T both live on SBUF
  partitions 64:128 so score matmuls satisfy the shared-base-partition
  rule; V tiles are PE-transposed into Vp [128k, 16, 65] with a ones
  column (softmax denominator). With zero biases the early PSUM->SBUF
  copies run on the otherwise-idle ACT engine.
  Attention over local key tiles kt, with score/exp/mask/PV regions
  trimmed to the causal need:
    kt 0..3  : cols [kt*128:1024] (slot0 tri tail + slot1 full), one exp
    kt 4..7  : slot1 tri tail only
    kt 8..11 : both slots; slot0 zeroed by a per-core 0/1 gate vector
    kt 12..15: slot1 only; per-core 0/-30000 exp bias kills it on h=1
  PV accumulates out^T [65, 512] per slot in PSUM (col 64 = sum exp);
  the host does the final divide + transpose + scatter.
"""

import sys

if "/opt/trn_rl_repo" not in sys.path:
    sys.path.insert(0, "/opt/trn_rl_repo")

import numpy as np

B, S, D, H = 4, 2048, 1024, 64
P = 128
KO = D // P          # 8 dmodel chunks
NT = S // P          # 16 seq tiles
NEG = -30000.0


def _halves():
    return [[(0, 512), (1536, 2048)], [(512, 1536)]]


def _build_program(zb):
    import concourse.bacc as bacc
    import concourse.mybir as mybir
    import concourse.tile as tile

    f32 = mybir.dt.float32
    bf16 = mybir.dt.bfloat16
    AF = mybir.ActivationFunctionType
    ALU = mybir.AluOpType

    nc = bacc.Bacc()
    # et layout [chunk, partition, KO*512]: 8 KB contiguous per partition
    # per chunk -> big DMA descriptors (1 KB descriptors run ~21 GB/s/queue)
    et = nc.declare_dram_parameter("et", [4, P, KO * 512], bf16, isOutput=False)
    # weights + ET chunk 0 fused into one DMA: per partition
    # cols 0:1536 = [Wv|Wk|Wq/8] x 8 ko (192 each), cols 1536:5632 = chunk0
    wc0 = nc.declare_dram_parameter("wc0", [P, 1536 + 4096], bf16, isOutput=False)
    # cols: bq/8 | bk | g8 | g12n | bv (bv only rows 0:64 meaningful)
    bias4 = nc.declare_dram_parameter("bias4", [P, 5], f32, isOutput=False)
    # cols 0:2048 = tri masks (4 x 512), cols 2048:2112 = identity (rows 0:64)
    mi = nc.declare_dram_parameter("mi", [P, 4 * 512 + H], bf16, isOutput=False)
    out = nc.declare_dram_parameter("out", [H + 1, 1024], f32, isOutput=True)

    from contextlib import ExitStack

    with tile.TileContext(nc) as tc, ExitStack() as ctx:
        cpool = ctx.enter_context(tc.tile_pool(name="const", bufs=1))
        vtp = ctx.enter_context(tc.tile_pool(name="vt", bufs=2))
        ptp = ctx.enter_context(tc.tile_pool(name="pt", bufs=10))
        psb = ctx.enter_context(tc.tile_pool(name="psb", bufs=2, space="PSUM"))

        # --- input DMAs. Each dma_start spreads across all 16 queues at
        # ~300 GB/s aggregate but pays ~0.5us fixed cost, and queues come
        # up staggered over the first ~7us - so: few dma_starts, the
        # startup-critical ones (weights, chunk0 sub-chunks) first.
        wc_sb = cpool.tile([P, 1536 + 4096], bf16, tag="wc0")
        # two halves: subtile deps let the first projection matmuls start
        # once the first half (weights + chunk0 ko 0..4) has landed
        nc.sync.dma_start(wc_sb[:, 0:4096], wc0[:, 0:4096])
        nc.sync.dma_start(wc_sb[:, 4096:5632], wc0[:, 4096:5632])
        # [partition, chunk, ko, 512]; chunk 0 lives in wc_sb instead
        ET = cpool.tile([P, 4, KO, 512], bf16, tag="ET")
        nc.sync.dma_start(ET[:, 1, :, :], et[1, :, :])

        def w_ap(ko, a, b):      # weight cols a:b of ko-th 192-block
            return wc_sb[:, ko * 192 + a:ko * 192 + b]

        def et_ap(cc, ko):       # ET chunk cc, ko-th 512-col block
            if cc == 0:
                return wc_sb[:, 1536 + ko * 512:1536 + (ko + 1) * 512]
            return ET[:, cc, ko, :]
        mi_sb = cpool.tile([P, 4 * 512 + H], bf16, tag="mi")
        nc.sync.dma_start(mi_sb[:], mi[:])
        nc.sync.dma_start(ET[:, 2, :, :], et[2, :, :])
        # biases/gates are first consumed around vk2's copies / kt8, so
        # this tiny dma slots between chunks 2 and 3 - early enough for
        # its readers, without its fixed issue cost delaying chunk 2
        bias_sb = cpool.tile([P, 5], f32, tag="bias4")
        nc.sync.dma_start(bias_sb[:], bias4[:])
        nc.sync.dma_start(ET[:, 3, :, :], et[3, :, :])
        bq_sb = bias_sb[:, 0:1]
        bk_sb = bias_sb[:, 1:2]
        g8_sb = bias_sb[:, 2:3]
        g12_sb = bias_sb[:, 3:4]
        bv_sb = bias_sb[:H, 4:5]
        id_sb = mi_sb[:H, 4 * 512:4 * 512 + H]

        # Q^T and K^T both live on partitions 64:128 (matmul requires lhsT
        # and rhs to share a base partition; the packed [Wv|Wk] projection
        # puts K^T on PSUM rows 64:128 and DVE copies cannot shift rows).
        QT = cpool.tile([P, 1024], bf16, tag="QT")
        KT = cpool.tile([P, S], bf16, tag="KT")
        Vp = cpool.tile([P, NT, H + 1], bf16, tag="Vp")
        nc.vector.memset(Vp[:, :, H:H + 1], 1.0)
        o_sb = cpool.tile([P, 1024], f32, tag="osb")

        def vtranspose(vt, cc):
            for t in range(4):
                kt = cc * 4 + t
                pvt = psb.tile([P, H], bf16, tag="pj", name=f"pvt_{kt}")
                nc.tensor.transpose(
                    pvt[:], vt[:, t * P:(t + 1) * P], id_sb[:]
                )
                nc.vector.tensor_copy(Vp[:, kt, :H], pvt[:])

        vts = [None] * 4

        def pcopy(dst, src_ap, bias, on_act):
            # PSUM->SBUF projection copy; with zero biases the early copies
            # run on the ACT engine, which is otherwise idle until the
            # first exp (~15us later)
            if zb and on_act:
                nc.scalar.activation(dst, src_ap, AF.Copy)
            else:
                nc.vector.tensor_scalar_add(dst, src_ap, bias)

        def vk_chunk(cc):
            # one pass of the ET chunk computes V^T (rows 0:64) + K^T (64:128)
            ps = psb.tile([P, 512], f32, tag="pj", name=f"vk_ps_{cc}")
            for ko in range(KO):
                nc.tensor.matmul(
                    ps[:], w_ap(ko, 0, 128), et_ap(cc, ko),
                    start=(ko == 0), stop=(ko == KO - 1),
                )
            on_act = cc < 2
            pcopy(
                KT[H:P, cc * 512:(cc + 1) * 512], ps[H:P, :], bk_sb[H:P],
                on_act,
            )
            vt = vtp.tile([H, 512], bf16, tag="vt", name=f"vt_{cc}")
            pcopy(vt[:], ps[:H, :], bv_sb[:], on_act)
            vts[cc] = vt

        def q_chunk(cc):
            # M=64 matmul targeting PSUM rows 64:128 so Q^T lands at base 64
            ps = psb.tile([P, 512], f32, tag="pj", name=f"q_ps_{cc}")
            for ko in range(KO):
                nc.tensor.matmul(
                    ps[H:P, :], w_ap(ko, 128, 192), et_ap(cc, ko),
                    start=(ko == 0), stop=(ko == KO - 1),
                )
            pcopy(
                QT[H:P, cc * 512:(cc + 1) * 512], ps[H:P, :], bq_sb[H:P],
                True,
            )

        # --- attention ---
        outT0 = psb.tile([P, 512], f32, tag="os0", bufs=1)
        outT1 = psb.tile([P, 512], f32, tag="os1", bufs=1)

        # pvs[kt] = list of (outT, col0, rhs_ap) PV pieces for that key tile
        pvs = [None] * NT

        def sc(kt):
            # score regions trimmed to the causal need:
            #  kt 0..3  : cols [kt*128 : 1024] (slot0 tri tail + slot1 full)
            #  kt 4..7  : slot1 tri tail, cols [(kt-4)*128 : 512] of slot1
            #  kt 8..11 : both slots full; slot0 multiplied by 0/1 gate
            #  kt 12..15: slot1 full, exp-bias gated
            ps = psb.tile(
                [P, 1024], f32, tag="sc", name=f"sc_{kt}", bufs=2
            )
            kblk = KT[H:P, kt * P:(kt + 1) * P]
            pt = ptp.tile([P, 1024], bf16, tag="pt", name=f"pt_{kt}")
            if kt < 4 or (8 <= kt < 12):
                c0 = kt * P if kt < 4 else 0
                nc.tensor.matmul(
                    ps[:, c0:512], kblk, QT[H:P, c0:512],
                    start=True, stop=True, skip_group_check=True,
                )
                nc.tensor.matmul(
                    ps[:, 512:1024], kblk, QT[H:P, 512:1024],
                    start=True, stop=True, skip_group_check=True,
                )
                nc.scalar.activation(pt[:, c0:1024], ps[:, c0:1024], AF.Exp)
                if kt < 4:
                    if c0 < 512:
                        nc.vector.tensor_tensor(
                            pt[:, c0:512], pt[:, c0:512],
                            mi_sb[:, kt * 512 + c0:(kt + 1) * 512], ALU.mult
                        )
                else:
                    nc.vector.tensor_scalar_mul(
                        pt[:, 0:512], pt[:, 0:512], g8_sb[:]
                    )
                pvs[kt] = [
                    (outT0, c0, pt[:, c0:512]),
                    (outT1, 0, pt[:, 512:1024]),
                ]
            else:
                c0 = (kt - 4) * P if kt < 12 else 0
                n = 512 - c0
                nc.tensor.matmul(
                    ps[:, 0:n], kblk, QT[H:P, 512 + c0:1024],
                    start=True, stop=True, skip_group_check=True,
                )
                if kt >= 12:
                    nc.scalar.activation(
                        pt[:, 0:n], ps[:, 0:n], AF.Exp, bias=g12_sb[:]
                    )
                else:
                    nc.scalar.activation(pt[:, 0:n], ps[:, 0:n], AF.Exp)
                    nc.vector.tensor_tensor(
                        pt[:, 0:n], pt[:, 0:n],
                        mi_sb[:, (kt - 4) * 512 + c0:(kt - 3) * 512], ALU.mult
                    )
                pvs[kt] = [(outT1, c0, pt[:, 0:n])]

        def pv(kt):
            for outT, c0, rhs in pvs[kt]:
                nc.tensor.matmul(
                    outT[:H + 1, c0:512], Vp[:, kt, :], rhs,
                    start=(kt == 0),
                    stop=(kt == 11 if outT is outT0 else kt == 15),
                    skip_group_check=True,
                )

        # --- emission order = per-engine FIFO order; hand-pipelined so PE
        # never waits on ACT/DVE and ACT starts exping early ---
        # HAM warmup: the PE sequencer comes alive ~4us before the first
        # input DMA lands; dependency-free matmuls on an uninitialized
        # scratch tile open the clock gate (1.2 -> 2.4 GHz) in that window
        # so the projections run warm. Results go to a dead psum tile.
        wtile = cpool.tile([P, P], bf16, tag="warm")
        nc.vector.memset(wtile[:], 0.0)
        for i in range(30):
            wps = psb.tile([P, H], f32, tag="pj", name=f"warm_{i}")
            nc.tensor.matmul(
                wps[:], wtile[:], wtile[:, 0:H],
                start=True, stop=True, skip_group_check=True,
            )

        # kt 0 and 1 split in half-scores: the slot0 halves (which need
        # only Q chunk 0) issue before q_chunk(1), so ACT starts exping
        # ~2us earlier in the proj->attention transition
        eps = {}
        ept = {}

        def sc_half_a(kt):
            c0 = kt * P
            ps = psb.tile([P, 1024], f32, tag="sc", name=f"sc_{kt}", bufs=2)
            pt = ptp.tile([P, 1024], bf16, tag="pt", name=f"pt_{kt}")
            eps[kt], ept[kt] = ps, pt
            nc.tensor.matmul(
                ps[:, c0:512], KT[H:P, kt * P:(kt + 1) * P], QT[H:P, c0:512],
                start=True, stop=True, skip_group_check=True,
            )
            nc.scalar.activation(pt[:, c0:512], ps[:, c0:512], AF.Exp)
            nc.vector.tensor_tensor(
                pt[:, c0:512], pt[:, c0:512],
                mi_sb[:, kt * 512 + c0:(kt + 1) * 512], ALU.mult
            )

        def sc_half_b(kt):
            ps, pt = eps[kt], ept[kt]
            nc.tensor.matmul(
                ps[:, 512:1024], KT[H:P, kt * P:(kt + 1) * P],
                QT[H:P, 512:1024],
                start=True, stop=True, skip_group_check=True,
            )
            nc.scalar.activation(pt[:, 512:1024], ps[:, 512:1024], AF.Exp)
            pvs[kt] = [
                (outT0, kt * P, pt[:, kt * P:512]),
                (outT1, 0, pt[:, 512:1024]),
            ]

        def sc67():
            # kt 6 (256 cols) and 7 (128 cols) share one psum bank + exp
            ps = psb.tile([P, 1024], f32, tag="sc", name="sc_67", bufs=2)
            pt = ptp.tile([P, 1024], bf16, tag="pt", name="pt_67")
            nc.tensor.matmul(
                ps[:, 0:256], KT[H:P, 6 * P:7 * P], QT[H:P, 768:1024],
                start=True, stop=True, skip_group_check=True,
            )
            nc.tensor.matmul(
                ps[:, 256:384], KT[H:P, 7 * P:8 * P], QT[H:P, 896:1024],
                start=True, stop=True, skip_group_check=True,
            )
            nc.scalar.activation(pt[:, 0:384], ps[:, 0:384], AF.Exp)
            nc.vector.tensor_tensor(
                pt[:, 0:256], pt[:, 0:256], mi_sb[:, 2 * 512 + 256:3 * 512],
                ALU.mult
            )
            nc.vector.tensor_tensor(
                pt[:, 256:384], pt[:, 256:384], mi_sb[:, 3 * 512 + 384:4 * 512],
                ALU.mult
            )
            pvs[6] = [(outT1, 256, pt[:, 0:256])]
            pvs[7] = [(outT1, 384, pt[:, 256:384])]

        vk_chunk(0)
        q_chunk(0)
        sc_half_a(0)
        sc_half_a(1)
        q_chunk(1)
        sc_half_b(0)
        sc_half_b(1)
        sc(2)
        sc(3)
        vk_chunk(1)
        sc(4)
        sc(5)
        sc67()
        vk_chunk(2)
        sc(8)
        sc(9)
        vtranspose(vts[0], 0)
        vtranspose(vts[1], 1)
        pv(0)
        pv(1)
        pv(2)
        pv(3)
        pv(4)
        pv(5)
        vk_chunk(3)
        vtranspose(vts[2], 2)
        sc(10)
        pv(6)
        sc(11)
        pv(7)
        sc(12)
        pv(8)
        sc(13)
        pv(9)
        vtranspose(vts[3], 3)
        sc(14)
        pv(10)
        sc(15)
        pv(11)
        nc.vector.tensor_copy(o_sb[:H + 1, 0:512], outT0[:H + 1, :])
        nc.sync.dma_start(out[:, 0:512], o_sb[:H + 1, 0:512])
        pv(12)
        pv(13)
        pv(14)
        pv(15)
        nc.vector.tensor_copy(o_sb[:H + 1, 512:1024], outT1[:H + 1, :])
        nc.sync.dma_start(out[:, 512:1024], o_sb[:H + 1, 512:1024])

    nc.finalize()
    return nc


_CACHED = None


def _get_program(zb):
    global _CACHED
    if _CACHED is None or _CACHED[0] != zb:
        _CACHED = (zb, _build_program(zb))
    return _CACHED[1]


def _host_inputs(embeddings, Wq, bq, Wk, bk, Wv, bv):
    import ml_dtypes

    bf16 = ml_dtypes.bfloat16
    halves = _halves()
    # multiplicative tri masks, [k, j, c] layout: 1 where c >= k + j*128
    masks = np.zeros((P, 4, 512), np.float32)
    for j in range(4):
        for k in range(P):
            masks[k, j, k + j * P:] = 1.0
    ident = np.zeros((P, H), np.float32)
    ident[:H] = np.eye(H, dtype=np.float32)
    mi = np.ascontiguousarray(
        np.concatenate([masks.reshape(P, 4 * 512), ident], axis=1)
    ).astype(bf16)

    def wlay(w):
        return np.asarray(w, np.float32).reshape(KO, P, H).transpose(1, 0, 2)

    wq8l = wlay(Wq) / 8.0
    wkl = wlay(Wk)
    wvl = wlay(Wv)
    wts = np.concatenate([wvl, wkl, wq8l], axis=2).reshape(P, 1536)
    bqf = np.asarray(bq, np.float32) / 8.0
    bkf = np.asarray(bk, np.float32)
    bvf = np.asarray(bv, np.float32)
    z64 = np.zeros(H, np.float32)
    bq8P = np.concatenate([z64, bqf])
    bkP = np.concatenate([z64, bkf])
    bvP = np.concatenate([bvf, z64])

    in_maps = []
    perms = []
    for c in range(8):
        b, h = c // 2, c % 2
        own = halves[h]
        other = halves[1 - h]
        rows = np.concatenate(
            [np.arange(a, z) for a, z in own] + [np.arange(a, z) for a, z in other]
        )
        perms.append(rows)
        ep = embeddings[b][rows]                      # [S, D] f32, permuted
        etl = np.ascontiguousarray(
            ep.T.reshape(KO, P, 4, 512).transpose(2, 1, 0, 3)
        ).astype(bf16).reshape(4, P, KO * 512)        # [cc, p, ko*512]
        g8v = np.full(P, 1.0 if h == 1 else 0.0, np.float32)
        g12v = np.full(P, NEG if h == 1 else 0.0, np.float32)
        bias4 = np.ascontiguousarray(
            np.stack([bq8P, bkP, g8v, g12v, bvP], axis=1)
        )
        wc0l = np.ascontiguousarray(
            np.concatenate([wts, etl[0]], axis=1)
        ).astype(bf16)
        in_maps.append({
            "et": etl, "wc0": wc0l, "bias4": bias4, "mi": mi,
        })
    return in_maps, perms


def _run(embeddings, Wq, bq, Wk, bk, Wv, bv, trace=False):
    from concourse.bass_utils import run_bass_kernel_spmd

    zb = (
        not np.any(np.asarray(bq)) and not np.any(np.asarray(bk))
        and not np.any(np.asarray(bv))
    )
    nc = _get_program(zb)
    in_maps, perms = _host_inputs(embeddings, Wq, bq, Wk, bk, Wv, bv)
    res = run_bass_kernel_spmd(
        nc, in_maps, core_ids=list(range(8)), trace=trace,
        trace_cores=list(range(8)) if trace else None,
    )
    full = np.empty((B, S, H), np.float32)
    for c in range(8):
        b = c // 2
        o = res.results[c]["out"]                     # [65, 1024] f32
        full[b, perms[c][:1024]] = (o[:H] / o[H:H + 1]).T
    return full, res


def kernel(embeddings, Wq, bq, Wk, bk, Wv, bv):
    full, _ = _run(
        np.asarray(embeddings, np.float32), Wq, bq, Wk, bk, Wv, bv, trace=False
    )
    return full


# revision 56
# speedup vs baseline: 1.0151x; 1.0007x over previous
"""Causal single-head attention on 8 trn2 NeuronCores.

B=4, S=2048, D_MODEL=1024, D_HEAD=64, fp32 in/out.

Sharding: 2 cores per batch. Core half h=0 owns query tiles {0..3,12..15}
(rows 0:512, 1536:2048), h=1 owns {4..11} (rows 512:1536); both own 68
causal 128x128 blocks. The host feeds each core its batch's embeddings
already TRANSPOSED to E^T [dm, s] in bf16 with columns permuted so own
query rows come first - no on-device transposes/casts of E at all.

Per-core pipeline (identical SPMD program, all matmuls bf16):
  Warmup matmuls on a scratch tile open the PE HAM clock gate
  (1.2->2.4 GHz) while the first input DMA is in flight. Inputs move in
  few large dma_starts (each internally spans all 16 DMA queues).
  Projections per 512-col chunk of E^T: one [Wv|Wk]-packed pass (V^T on
  PSUM rows 0:64, K^T on rows 64:128) plus, for the core's own 2 chunks,
  a Wq/8 pass targeting PSUM rows 64:128. Q^T and K^T both live on SBUF
  partitions 64:128 so score matmuls satisfy the shared-base-partition
  rule; V tiles are PE-transposed into Vp [128k, 16, 65] with a ones
  column (softmax denominator). With zero biases the early PSUM->SBUF
  copies run on the otherwise-idle ACT engine.
  Attention over local key tiles kt, with score/exp/mask/PV regions
  trimmed to the causal need:
    kt 0..3  : cols [kt*128:1024] (slot0 tri tail + slot1 full), one exp
    kt 4..7  : slot1 tri tail only
    kt 8..11 : both slots; slot0 zeroed by a per-core 0/1 gate vector
    kt 12..15: slot1 only; per-core 0/-30000 exp bias kills it on h=1
  PV accumulates out^T [65, 512] per slot in PSUM (col 64 = sum exp);
  the host does the final divide + transpose + scatter.
"""

import sys

if "/opt/trn_rl_repo" not in sys.path:
    sys.path.insert(0, "/opt/trn_rl_repo")

import numpy as np

B, S, D, H = 4, 2048, 1024, 64
P = 128
KO = D // P          # 8 dmodel chunks
NT = S // P          # 16 seq tiles
NEG = -30000.0


def _halves():
    return [[(0, 512), (1536, 2048)], [(512, 1536)]]


def _build_program(zb):
    import concourse.bacc as bacc
    import concourse.mybir as mybir
    import concourse.tile as tile

    f32 = mybir.dt.float32
    bf16 = mybir.dt.bfloat16
    AF = mybir.ActivationFunctionType
    ALU = mybir.AluOpType

    nc = bacc.Bacc()
    # et layout [chunk, partition, KO*512]: 8 KB contiguous per partition
    # per chunk -> big DMA descriptors (1 KB descriptors run ~21 GB/s/queue)
    et = nc.declare_dram_parameter("et", [4, P, KO * 512], bf16, isOutput=False)
    # weights + ET chunk 0 fused into one DMA: per partition
    # cols 0:1536 = [Wv|Wk|Wq/8] x 8 ko (192 each), cols 1536:5632 = chunk0
    wc0 = nc.declare_dram_parameter("wc0", [P, 1536 + 4096], bf16, isOutput=False)
    # cols: bq/8 | bk | g8 | g12n | bv (bv only rows 0:64 meaningful)
    bias4 = nc.declare_dram_parameter("bias4", [P, 5], f32, isOutput=False)
    # cols 0:2048 = tri masks (4 x 512), cols 2048:2112 = identity (rows 0:64)
    mi = nc.declare_dram_parameter("mi", [P, 4 * 512 + H], bf16, isOutput=False)
    out = nc.declare_dram_parameter("out", [P, 1024], f32, isOutput=True)

    from contextlib import ExitStack

    with tile.TileContext(nc) as tc, ExitStack() as ctx:
        cpool = ctx.enter_context(tc.tile_pool(name="const", bufs=1))
        vtp = ctx.enter_context(tc.tile_pool(name="vt", bufs=2))
        ptp = ctx.enter_context(tc.tile_pool(name="pt", bufs=10))
        psb = ctx.enter_context(tc.tile_pool(name="psb", bufs=2, space="PSUM"))

        # --- input DMAs. Each dma_start spreads across all 16 queues at
        # ~300 GB/s aggregate but pays ~0.5us fixed cost, and queues come
        # up staggered over the first ~7us - so: few dma_starts, the
        # startup-critical ones (weights, chunk0 sub-chunks) first.
        wc_sb = cpool.tile([P, 1536 + 4096], bf16, tag="wc0")
        # two halves: subtile deps let the first projection matmuls start
        # once the first half (weights + chunk0 ko 0..4) has landed
        nc.sync.dma_start(wc_sb[:, 0:4096], wc0[:, 0:4096])
        nc.sync.dma_start(wc_sb[:, 4096:5632], wc0[:, 4096:5632])
        # [partition, chunk, ko, 512]; chunk 0 lives in wc_sb instead
        ET = cpool.tile([P, 4, KO, 512], bf16, tag="ET")
        nc.sync.dma_start(ET[:, 1, :, :], et[1, :, :])

        def w_ap(ko, a, b):      # weight cols a:b of ko-th 192-block
            return wc_sb[:, ko * 192 + a:ko * 192 + b]

        def et_ap(cc, ko):       # ET chunk cc, ko-th 512-col block
            if cc == 0:
                return wc_sb[:, 1536 + ko * 512:1536 + (ko + 1) * 512]
            return ET[:, cc, ko, :]
        mi_sb = cpool.tile([P, 4 * 512 + H], bf16, tag="mi")
        nc.sync.dma_start(mi_sb[:], mi[:])
        nc.sync.dma_start(ET[:, 2, :, :], et[2, :, :])
        # biases/gates are first consumed around vk2's copies / kt8, so
        # this tiny dma slots between chunks 2 and 3 - early enough for
        # its readers, without its fixed issue cost delaying chunk 2
        bias_sb = cpool.tile([P, 5], f32, tag="bias4")
        nc.sync.dma_start(bias_sb[:], bias4[:])
        nc.sync.dma_start(ET[:, 3, :, :], et[3, :, :])
        bq_sb = bias_sb[:, 0:1]
        bk_sb = bias_sb[:, 1:2]
        g8_sb = bias_sb[:, 2:3]
        g12_sb = bias_sb[:, 3:4]
        bv_sb = bias_sb[:H, 4:5]
        id_sb = mi_sb[:H, 4 * 512:4 * 512 + H]

        # Q^T and K^T both live on partitions 64:128 (matmul requires lhsT
        # and rhs to share a base partition; the packed [Wv|Wk] projection
        # puts K^T on PSUM rows 64:128 and DVE copies cannot shift rows).
        QT = cpool.tile([P, 1024], bf16, tag="QT")
        KT = cpool.tile([P, S], bf16, tag="KT")
        Vp = cpool.tile([P, NT, H + 1], bf16, tag="Vp")
        nc.vector.memset(Vp[:, :, H:H + 1], 1.0)
        o_sb = cpool.tile([P, 1024], f32, tag="osb")
        nc.vector.memset(o_sb[H:P, :], 0.0)

        def vtranspose(vt, cc):
            for t in range(4):
                kt = cc * 4 + t
                pvt = psb.tile([P, H], bf16, tag="pj", name=f"pvt_{kt}")
                nc.tensor.transpose(
                    pvt[:], vt[:, t * P:(t + 1) * P], id_sb[:]
                )
                nc.vector.tensor_copy(Vp[:, kt, :H], pvt[:])

        vts = [None] * 4

        def pcopy(dst, src_ap, bias, on_act):
            # PSUM->SBUF projection copy; with zero biases the early copies
            # run on the ACT engine, which is otherwise idle until the
            # first exp (~15us later)
            if zb and on_act:
                nc.scalar.activation(dst, src_ap, AF.Copy)
            else:
                nc.vector.tensor_scalar_add(dst, src_ap, bias)

        def vk_chunk(cc):
            # one pass of the ET chunk computes V^T (rows 0:64) + K^T (64:128)
            ps = psb.tile([P, 512], f32, tag="pj", name=f"vk_ps_{cc}")
            for ko in range(KO):
                nc.tensor.matmul(
                    ps[:], w_ap(ko, 0, 128), et_ap(cc, ko),
                    start=(ko == 0), stop=(ko == KO - 1),
                )
            on_act = cc < 2
            pcopy(
                KT[H:P, cc * 512:(cc + 1) * 512], ps[H:P, :], bk_sb[H:P],
                on_act,
            )
            vt = vtp.tile([H, 512], bf16, tag="vt", name=f"vt_{cc}")
            pcopy(vt[:], ps[:H, :], bv_sb[:], on_act)
            vts[cc] = vt

        def q_chunk(cc):
            # M=64 matmul targeting PSUM rows 64:128 so Q^T lands at base 64
            ps = psb.tile([P, 512], f32, tag="pj", name=f"q_ps_{cc}")
            for ko in range(KO):
                nc.tensor.matmul(
                    ps[H:P, :], w_ap(ko, 128, 192), et_ap(cc, ko),
                    start=(ko == 0), stop=(ko == KO - 1),
                )
            pcopy(
                QT[H:P, cc * 512:(cc + 1) * 512], ps[H:P, :], bq_sb[H:P],
                False,
            )

        # --- attention ---
        outT0 = psb.tile([P, 512], f32, tag="os0", bufs=1)
        outT1 = psb.tile([P, 512], f32, tag="os1", bufs=1)

        # pvs[kt] = list of (outT, col0, rhs_ap) PV pieces for that key tile
        pvs = [None] * NT

        def sc(kt):
            # score regions trimmed to the causal need:
            #  kt 0..3  : cols [kt*128 : 1024] (slot0 tri tail + slot1 full)
            #  kt 4..7  : slot1 tri tail, cols [(kt-4)*128 : 512] of slot1
            #  kt 8..11 : both slots full; slot0 multiplied by 0/1 gate
            #  kt 12..15: slot1 full, exp-bias gated
            ps = psb.tile(
                [P, 1024], f32, tag="sc", name=f"sc_{kt}", bufs=2
            )
            kblk = KT[H:P, kt * P:(kt + 1) * P]
            pt = ptp.tile([P, 1024], bf16, tag="pt", name=f"pt_{kt}")
            if kt < 4 or (8 <= kt < 12):
                c0 = kt * P if kt < 4 else 0
                nc.tensor.matmul(
                    ps[:, c0:512], kblk, QT[H:P, c0:512],
                    start=True, stop=True, skip_group_check=True,
                )
                nc.tensor.matmul(
                    ps[:, 512:1024], kblk, QT[H:P, 512:1024],
                    start=True, stop=True, skip_group_check=True,
                )
                nc.scalar.activation(pt[:, c0:1024], ps[:, c0:1024], AF.Exp)
                if kt < 4:
                    if c0 < 512:
                        nc.vector.tensor_tensor(
                            pt[:, c0:512], pt[:, c0:512],
                            mi_sb[:, kt * 512 + c0:(kt + 1) * 512], ALU.mult
                        )
                else:
                    nc.vector.tensor_scalar_mul(
                        pt[:, 0:512], pt[:, 0:512], g8_sb[:]
                    )
                pvs[kt] = [
                    (outT0, c0, pt[:, c0:512]),
                    (outT1, 0, pt[:, 512:1024]),
                ]
            else:
                c0 = (kt - 4) * P if kt < 12 else 0
                n = 512 - c0
                nc.tensor.matmul(
                    ps[:, 0:n], kblk, QT[H:P, 512 + c0:1024],
                    start=True, stop=True, skip_group_check=True,
                )
                if kt >= 12:
                    nc.scalar.activation(
                        pt[:, 0:n], ps[:, 0:n], AF.Exp, bias=g12_sb[:]
                    )
                else:
                    nc.scalar.activation(pt[:, 0:n], ps[:, 0:n], AF.Exp)
                    nc.vector.tensor_tensor(
                        pt[:, 0:n], pt[:, 0:n],
                        mi_sb[:, (kt - 4) * 512 + c0:(kt - 3) * 512], ALU.mult
                    )
                pvs[kt] = [(outT1, c0, pt[:, 0:n])]

        def pv(kt):
            for outT, c0, rhs in pvs[kt]:
                nc.tensor.matmul(
                    outT[:H + 1, c0:512], Vp[:, kt, :], rhs,
                    start=(kt == 0),
                    stop=(kt == 11 if outT is outT0 else kt == 15),
                    skip_group_check=True,
                )

        # --- emission order = per-engine FIFO order; hand-pipelined so PE
        # never waits on ACT/DVE and ACT starts exping early ---
        # HAM warmup: the PE sequencer comes alive ~4us before the first
        # input DMA lands; dependency-free matmuls on an uninitialized
        # scratch tile open the clock gate (1.2 -> 2.4 GHz) in that window
        # so the projections run warm. Results go to a dead psum tile.
        wtile = cpool.tile([P, P], bf16, tag="warm")
        nc.vector.memset(wtile[:], 0.0)
        for i in range(30):
            wps = psb.tile([P, H], f32, tag="pj", name=f"warm_{i}")
            nc.tensor.matmul(
                wps[:], wtile[:], wtile[:, 0:H],
                start=True, stop=True, skip_group_check=True,
            )

        # kt 0 and 1 split in half-scores: the slot0 halves (which need
        # only Q chunk 0) issue before q_chunk(1), so ACT starts exping
        # ~2us earlier in the proj->attention transition
        eps = {}
        ept = {}

        def sc_half_a(kt):
            c0 = kt * P
            ps = psb.tile([P, 1024], f32, tag="sc", name=f"sc_{kt}", bufs=2)
            pt = ptp.tile([P, 1024], bf16, tag="pt", name=f"pt_{kt}")
            eps[kt], ept[kt] = ps, pt
            nc.tensor.matmul(
                ps[:, c0:512], KT[H:P, kt * P:(kt + 1) * P], QT[H:P, c0:512],
                start=True, stop=True, skip_group_check=True,
            )
            nc.scalar.activation(pt[:, c0:512], ps[:, c0:512], AF.Exp)
            nc.vector.tensor_tensor(
                pt[:, c0:512], pt[:, c0:512],
                mi_sb[:, kt * 512 + c0:(kt + 1) * 512], ALU.mult
            )

        def sc_half_b(kt):
            ps, pt = eps[kt], ept[kt]
            nc.tensor.matmul(
                ps[:, 512:1024], KT[H:P, kt * P:(kt + 1) * P],
                QT[H:P, 512:1024],
                start=True, stop=True, skip_group_check=True,
            )
            nc.scalar.activation(pt[:, 512:1024], ps[:, 512:1024], AF.Exp)
            pvs[kt] = [
                (outT0, kt * P, pt[:, kt * P:512]),
                (outT1, 0, pt[:, 512:1024]),
            ]

        def sc67():
            # kt 6 (256 cols) and 7 (128 cols) share one psum bank + exp
            ps = psb.tile([P, 1024], f32, tag="sc", name="sc_67", bufs=2)
            pt = ptp.tile([P, 1024], bf16, tag="pt", name="pt_67")
            nc.tensor.matmul(
                ps[:, 0:256], KT[H:P, 6 * P:7 * P], QT[H:P, 768:1024],
                start=True, stop=True, skip_group_check=True,
            )
            nc.tensor.matmul(
                ps[:, 256:384], KT[H:P, 7 * P:8 * P], QT[H:P, 896:1024],
                start=True, stop=True, skip_group_check=True,
            )
            nc.scalar.activation(pt[:, 0:384], ps[:, 0:384], AF.Exp)
            nc.vector.tensor_tensor(
                pt[:, 0:256], pt[:, 0:256], mi_sb[:, 2 * 512 + 256:3 * 512],
                ALU.mult
            )
            nc.vector.tensor_tensor(
                pt[:, 256:384], pt[:, 256:384], mi_sb[:, 3 * 512 + 384:4 * 512],
                ALU.mult
            )
            pvs[6] = [(outT1, 256, pt[:, 0:256])]
            pvs[7] = [(outT1, 384, pt[:, 256:384])]

        vk_chunk(0)
        q_chunk(0)
        sc_half_a(0)
        sc_half_a(1)
        q_chunk(1)
        sc_half_b(0)
        sc_half_b(1)
        sc(2)
        sc(3)
        vk_chunk(1)
        sc(4)
        sc(5)
        sc67()
        vk_chunk(2)
        sc(8)
        sc(9)
        vtranspose(vts[0], 0)
        vtranspose(vts[1], 1)
        pv(0)
        pv(1)
        pv(2)
        pv(3)
        pv(4)
        pv(5)
        vk_chunk(3)
        vtranspose(vts[2], 2)
        sc(10)
        pv(6)
        sc(11)
        pv(7)
        sc(12)
        pv(8)
        sc(13)
        pv(9)
        vtranspose(vts[3], 3)
        sc(14)
        pv(10)
        sc(15)
        pv(11)
        nc.vector.tensor_copy(o_sb[:H + 1, 0:512], outT0[:H + 1, :])
        nc.sync.dma_start(out[:, 0:512], o_sb[:, 0:512])
        pv(12)
        pv(13)
        pv(14)
        pv(15)
        nc.vector.tensor_copy(o_sb[:H + 1, 512:1024], outT1[:H + 1, :])
        nc.sync.dma_start(out[:, 512:1024], o_sb[:, 512:1024])

    nc.finalize()
    return nc


_CACHED = None


def _get_program(zb):
    global _CACHED
    if _CACHED is None or _CACHED[0] != zb:
        _CACHED = (zb, _build_program(zb))
    return _CACHED[1]


def _host_inputs(embeddings, Wq, bq, Wk, bk, Wv, bv):
    import ml_dtypes

    bf16 = ml_dtypes.bfloat16
    halves = _halves()
    # multiplicative tri masks, [k, j, c] layout: 1 where c >= k + j*128
    masks = np.zeros((P, 4, 512), np.float32)
    for j in range(4):
        for k in range(P):
            masks[k, j, k + j * P:] = 1.0
    ident = np.zeros((P, H), np.float32)
    ident[:H] = np.eye(H, dtype=np.float32)
    mi = np.ascontiguousarray(
        np.concatenate([masks.reshape(P, 4 * 512), ident], axis=1)
    ).astype(bf16)

    def wlay(w):
        return np.asarray(w, np.float32).reshape(KO, P, H).transpose(1, 0, 2)

    wq8l = wlay(Wq) / 8.0
    wkl = wlay(Wk)
    wvl = wlay(Wv)
    wts = np.concatenate([wvl, wkl, wq8l], axis=2).reshape(P, 1536)
    bqf = np.asarray(bq, np.float32) / 8.0
    bkf = np.asarray(bk, np.float32)
    bvf = np.asarray(bv, np.float32)
    z64 = np.zeros(H, np.float32)
    bq8P = np.concatenate([z64, bqf])
    bkP = np.concatenate([z64, bkf])
    bvP = np.concatenate([bvf, z64])

    in_maps = []
    perms = []
    for c in range(8):
        b, h = c // 2, c % 2
        own = halves[h]
        other = halves[1 - h]
        rows = np.concatenate(
            [np.arange(a, z) for a, z in own] + [np.arange(a, z) for a, z in other]
        )
        perms.append(rows)
        ep = embeddings[b][rows]                      # [S, D] f32, permuted
        etl = np.ascontiguousarray(
            ep.T.reshape(KO, P, 4, 512).transpose(2, 1, 0, 3)
        ).astype(bf16).reshape(4, P, KO * 512)        # [cc, p, ko*512]
        g8v = np.full(P, 1.0 if h == 1 else 0.0, np.float32)
        g12v = np.full(P, NEG if h == 1 else 0.0, np.float32)
        bias4 = np.ascontiguousarray(
            np.stack([bq8P, bkP, g8v, g12v, bvP], axis=1)
        )
        wc0l = np.ascontiguousarray(
            np.concatenate([wts, etl[0]], axis=1)
        ).astype(bf16)
        in_maps.append({
            "et": etl, "wc0": wc0l, "bias4": bias4, "mi": mi,
        })
    return in_maps, perms


def _run(embeddings, Wq, bq, Wk, bk, Wv, bv, trace=False):
    from concourse.bass_utils import run_bass_kernel_spmd

    zb = (
        not np.any(np.asarray(bq)) and not np.any(np.asarray(bk))
        and not np.any(np.asarray(bv))
    )
    nc = _get_program(zb)
    in_maps, perms = _host_inputs(embeddings, Wq, bq, Wk, bk, Wv, bv)
    res = run_bass_kernel_spmd(
        nc, in_maps, core_ids=list(range(8)), trace=trace,
        trace_cores=list(range(8)) if trace else None,
    )
    full = np.empty((B, S, H), np.float32)
    for c in range(8):
        b = c // 2
        o = res.results[c]["out"][:H + 1]             # [65, 1024] f32
        full[b, perms[c][:1024]] = (o[:H] / o[H:H + 1]).T
    return full, res


def kernel(embeddings, Wq, bq, Wk, bk, Wv, bv):
    full, _ = _run(
        np.asarray(embeddings, np.float32), Wq, bq, Wk, bk, Wv, bv, trace=False
    )
    return full
